# revision 26
# baseline (speedup 1.0000x reference)
"""Top-1 MoE (BmmMoeModel) on 8 Trainium2 NeuronCores.

Strategy: expert-parallel with routing-aware dispatch.
  - Host computes the router (x @ gate_w.T, argmax, sigmoid) -- 0.26% of the
    model FLOPs -- and uses it as the sharding function: each core receives
    only the tokens routed to its expert (scaled by the routing weight,
    transposed to [H, C], cast to bf16) plus that expert's weights.
  - The final "sum over experts" is a disjoint scatter of each core's token
    outputs back into the full [T, H] output on the host (top-1 routing means
    non-selected experts contribute exactly zero).

On-device schedule (all matmuls bf16, fp32 PSUM, stationary = activations,
moving = weights; at N=512 the PE streams 213 ns/matmul and LDWEIGHTS is
fully hidden by the reorder window, so the only wins left are at the edges):

  Phase A runs in TWO PASSES over (up, gate) column pairs.  Pass p streams
  weight columns [p*512:(p+1)*512] (up) and [1024+p*512:...] (gate) of every
  k-tile, pre-paired by the host into one contiguous [128, 1024] tile per
  (pass, k).  All four 128-token c-chunks accumulate over the FULL k=16
  contraction in PSUM (4c x 2 halves = 8 banks), so there is no partial-sum
  spill/merge at all, and the silu-glu fuses straight out of PSUM:
  act[c, pass-cols] = up * silu(gate) (one scalar.activation + one
  vector.mul).  Per arriving 0.25 MB weight tile the PE does 1.7 us of work
  vs 0.61 us arrival time, so the PE never starves once started (the old
  half-k/spill schedule consumed 1.7 us per 2.56-us-arriving tile during its
  first c-pair pass and idled ~5 us).

  act is transposed per pass to i-major actT with the DMA xbar (off the
  critical engines).  Phase B (down proj) is token-stationary: stationary =
  actT[i-chunk, c-chunk], moving = wdn[i-chunk, :] in 4 q-slices of 512,
  PSUM accumulates over i; output lands token-major [C, H] in bf16 (halves
  the drain tail; host upcasts).  The last c-chunk runs banks q0-q2 first so
  the post-final-matmul tail is a single 512-wide cast + 0.25 MB DMA.

DMA plan (measured: the NEFF preamble releases SWDGE/gpsimd at ~4.2 us but
HWDGE issuers only at ~7.0 us, and each dma_start occupies its issuing
engine ~0.6 us): the PE-gating pieces (xsA = k-tile 0 of tokens, the two
512-wide halves of pass-0 k-tile 0, xsB, pass-0 k-tiles 1-2) issue from
GPSIMD starting ~4.2 us, everything else from SCALAR in consumption order;
outputs ride sync/gpsimd.  The PE warmup spin (HAM clock gate holds the PE
at 1.2 GHz until ~3.4 us of sustained activity) is sized to end right when
the first weights land.
"""

import numpy as np
import ml_dtypes

B, S, H, I, E = 2, 2048, 2048, 1024, 8
T = B * S
FF2 = 2 * I
C = 512          # per-expert device token capacity (4 c-chunks of 128)
KH = H // 128    # 16 contraction chunks for gate_up
KI = I // 128    # 8 contraction chunks for down

# Stash of the last run's BassKernelResults (for test harness introspection).
LAST = {}
_PROGRAM_CACHE = {}


def _build_program():
    import concourse.bass as bass
    import concourse.mybir as mybir
    import concourse.tile as tile
    from concourse import bacc

    dt = mybir.dt
    AF = mybir.ActivationFunctionType

    nc = bacc.Bacc(None, target_bir_lowering=False)
    # xsP[p, k*C + c] = x_tokens[c, k*128 + p]: partition-major so each DMA
    # row run is 4 KB (k-group of 4) instead of 1 KB.
    xsP = nc.dram_tensor("xsP", [128, KH * C], dt.bfloat16,
                         kind="ExternalInput")
    # Packed PE-gating pieces (every dma_start costs ~0.65 us of
    # issuing-engine time and each tile completion pays ~0.5-1 us of
    # semaphore receipt, so the head wants FEW, SMALL, need-ordered pieces):
    #   xsW0 = [k-tile 0 of tokens | pass-0 k0 up cols]   -> first 4 matmuls
    #   wg0b = [pass-0 k0 gate cols]                      -> next 4
    #   xsW1 = [k-tiles 1-3 of tokens | pass-0 k1]        -> k=1 group
    xsW0 = nc.dram_tensor("xsW0", [128, 2 * 512], dt.bfloat16,
                          kind="ExternalInput")
    wg0b = nc.dram_tensor("wg0b", [128, 512], dt.bfloat16,
                          kind="ExternalInput")
    xsW1 = nc.dram_tensor("xsW1", [128, 5 * 512], dt.bfloat16,
                          kind="ExternalInput")
    # Pass-paired gate_up weights (host pre-arranged):
    #   wguP0[k, p, :512]  = W[k-tile, p,  q0-cols]   (up,   pass 0)
    #   wguP0[k, p, 512:]  = W[k-tile, p,  q2-cols]   (gate, pass 0)
    # wguP1 is the same for pass 1 (q1/q3) but partition-major so pass 1
    # streams as four 1 MB DMAs with 8 KB contiguous rows.
    wguP0 = nc.dram_tensor("wguP0", [KH, 128, 1024], dt.bfloat16,
                           kind="ExternalInput")
    wguP1 = nc.dram_tensor("wguP1", [128, KH * 1024], dt.bfloat16,
                           kind="ExternalInput")
    wdn = nc.dram_tensor("wdn", [I, H], dt.bfloat16, kind="ExternalInput")
    # Output in bf16: halves the output HBM traffic and the post-matmul
    # drain tail; the host upcasts (adds ~0.2% RMS, far inside the gate).
    outC = nc.dram_tensor("outC", [C, H], dt.bfloat16, kind="ExternalOutput")

    with tile.TileContext(nc) as tc:
        with (
            tc.tile_pool(name="res", bufs=1) as res,
            tc.tile_pool(name="work", bufs=1) as work,
            tc.tile_pool(name="psum", bufs=1, space=bass.MemorySpace.PSUM) as psum,
        ):
            # PE clock pre-warm (see module docstring).
            warm = work.tile([128, 128], dt.bfloat16, tag="warm", bufs=1,
                             name="warm")
            nc.vector.memset(warm[:], 0.0)
            warm_ps = psum.tile([128, 128], dt.float32, tag="ps", bufs=8,
                                name="warm_ps")
            for _ in range(25):
                nc.tensor.matmul(warm_ps[:], warm[:], warm[:],
                                 start=True, stop=True)

            pk0 = res.tile([128, 2, 512], dt.bfloat16, tag="pk0", name="pk0")
            pk0b = res.tile([128, 512], dt.bfloat16, tag="pk0b", name="pk0b")
            pk1 = res.tile([128, 5, 512], dt.bfloat16, tag="pk1", name="pk1")
            xs4 = [None] + [res.tile([128, 4, C], dt.bfloat16, tag=f"xs{b}",
                                     name=f"xs{b}") for b in range(1, 4)]
            wgp0 = [None, None] + [
                res.tile([128, 1024], dt.bfloat16, tag=f"wgp0{k}",
                         name=f"wgp0{k}") for k in range(2, KH)]
            wgp1 = [res.tile([128, 4, 1024], dt.bfloat16, tag=f"wgp1{b}",
                             name=f"wgp1{b}") for b in range(4)]
            xsP_r = xsP.rearrange("p (b j c) -> b p j c", b=4, j=4)
            xsW0_r = xsW0.rearrange("p (j c) -> p j c", j=2)
            xsW1_r = xsW1.rearrange("p (j c) -> p j c", j=5)
            wguP1_r = wguP1.rearrange("p (b j f) -> b p j f", b=4, j=4)

            # ONE input stream on SCALAR (HWDGE), in exact consumption order
            # (strict FIFO keeps arrival == need order at full bandwidth; a
            # second concurrent input queue just splits bandwidth between
            # tiles needed now and tiles needed later).
            nc.scalar.dma_start(pk0[:], xsW0_r)
            nc.scalar.dma_start(pk0b[:], wg0b[0:128, :])
            nc.scalar.dma_start(pk1[:], xsW1_r)
            nc.scalar.dma_start(wgp0[2][:], wguP0[2])
            nc.scalar.dma_start(xs4[1][:], xsP_r[1])
            for k in range(3, 8):
                nc.scalar.dma_start(wgp0[k][:], wguP0[k])
            nc.scalar.dma_start(xs4[2][:], xsP_r[2])
            nc.scalar.dma_start(xs4[3][:], xsP_r[3])
            for k in range(8, KH):
                nc.scalar.dma_start(wgp0[k][:], wguP0[k])
            for b in range(4):
                nc.scalar.dma_start(wgp1[b][:], wguP1_r[b])
            # down weight (Phase B moving): 2 tiles of 4 i-chunks each.
            wd4 = []
            wdn_r = wdn.rearrange("(b j p) h -> b p j h", j=4, p=128)
            for b in range(2):
                t_ = res.tile([128, 4, H], dt.bfloat16, tag=f"wd{b}",
                              name=f"wd{b}")
                nc.scalar.dma_start(t_[:], wdn_r[b])
                wd4.append(t_)

            def xs_sl(k, c):
                if k == 0:
                    return pk0[:, 0, c * 128:(c + 1) * 128]
                if k < 4:
                    return pk1[:, k - 1, c * 128:(c + 1) * 128]
                return xs4[k // 4][:, k % 4, c * 128:(c + 1) * 128]

            act = [res.tile([128, I], dt.bfloat16, tag=f"act{c}",
                            name=f"act{c}") for c in range(4)]
            # actT[c][:, j, :] = act[c][:, j*128:(j+1)*128].T  (i-major)
            actT = [res.tile([128, KI, 128], dt.bfloat16, tag=f"actT{c}",
                             name=f"actT{c}") for c in range(4)]

            # Phase A: two passes, full-k PSUM accumulation, fused silu-glu.
            # The last SG k-steps run per-c so each chunk's consume (the
            # bank-freeing silu+mul) overlaps the next chunk's k-tail.
            SG = 3
            for ps in range(2):
                pa = {(c, h): psum.tile([128, 512], dt.float32, tag="ps",
                                        bufs=8, name=f"pa{ps}_{c}_{h}")
                      for c in range(4) for h in range(2)}

                def wg_sl(k, h, ps=ps):
                    if ps == 0:
                        if k == 0:
                            return pk0[:, 1, :] if h == 0 else pk0b[:]
                        if k == 1:
                            return pk1[:, 3 + h, :]
                        return wgp0[k][:, h * 512:(h + 1) * 512]
                    return wgp1[k // 4][:, k % 4, h * 512:(h + 1) * 512]

                def a_mm(k, c, pa=pa, wg_sl=wg_sl):
                    lhs = xs_sl(k, c)
                    for h in range(2):
                        nc.tensor.matmul(
                            pa[(c, h)][:], lhs, wg_sl(k, h),
                            start=(k == 0), stop=(k == KH - 1),
                        )

                def consume(c, pa=pa, ps=ps):
                    st = work.tile([128, 512], dt.float32, tag="silu",
                                   bufs=4, name=f"st{ps}_{c}")
                    nc.scalar.activation(st[:], pa[(c, 1)][:], AF.Silu)
                    nc.vector.tensor_mul(
                        act[c][:, ps * 512:(ps + 1) * 512],
                        pa[(c, 0)][:], st[:])
                    nc.sync.dma_start_transpose(
                        actT[c][:, ps * 4:(ps + 1) * 4, :],
                        act[c][:, ps * 512:(ps + 1) * 512])

                for k in range(0, KH - SG):
                    if k == 0:
                        # h0 for all c first: the h1 weight piece is still
                        # in flight when the first matmuls start.
                        for h in range(2):
                            for c in range(4):
                                nc.tensor.matmul(
                                    pa[(c, h)][:], xs_sl(0, c), wg_sl(0, h),
                                    start=True, stop=False)
                    else:
                        for c in range(4):
                            a_mm(k, c)
                for c in range(4):
                    for k in range(KH - SG, KH):
                        a_mm(k, c)
                    consume(c)

            # Phase B: token-major out, one c-chunk at a time (4 banks).
            for c in range(4):
                if c < 3:
                    pb = [psum.tile([128, 512], dt.float32, tag="ps", bufs=8,
                                    name=f"pb{c}_{q}") for q in range(4)]
                    for j in range(KI):
                        lhs = actT[c][:, j, :]
                        for q in range(4):
                            nc.tensor.matmul(
                                pb[q][:], lhs,
                                wd4[j // 4][:, j % 4, q * 512:(q + 1) * 512],
                                start=(j == 0), stop=(j == KI - 1),
                            )
                    ot = work.tile([128, H], dt.bfloat16, tag="ot", bufs=2,
                                   name=f"ot{c}")
                    for q in range(4):
                        nc.vector.tensor_copy(ot[:, q * 512:(q + 1) * 512],
                                              pb[q][:])
                    # alternate output queues so the final drains overlap
                    eng = nc.gpsimd if c % 2 == 0 else nc.sync
                    eng.dma_start(outC[c * 128:(c + 1) * 128, :], ot[:])
                else:
                    # banks q0-q2 interleaved (stationary reused), then the
                    # last 512 columns in two pieces (384 + 128): q0-q2's
                    # casts+DMAs overlap the 384-piece's matmuls, the
                    # 384-piece's drain overlaps the 128-piece's matmuls,
                    # and the post-final-matmul tail is one 64 KB DMA.
                    pb = [psum.tile([128, 512], dt.float32, tag="ps", bufs=8,
                                    name=f"pb{c}_{q}") for q in range(3)]
                    pb3a = psum.tile([128, 384], dt.float32, tag="ps", bufs=8,
                                     name=f"pb{c}_3a")
                    pb3b = psum.tile([128, 128], dt.float32, tag="ps", bufs=8,
                                     name=f"pb{c}_3b")
                    for j in range(KI):
                        lhs = actT[c][:, j, :]
                        for q in range(3):
                            nc.tensor.matmul(
                                pb[q][:], lhs,
                                wd4[j // 4][:, j % 4, q * 512:(q + 1) * 512],
                                start=(j == 0), stop=(j == KI - 1),
                            )
                    for q in range(3):
                        otq = work.tile([128, 512], dt.bfloat16, tag="otl",
                                        bufs=4, name=f"ot{c}_{q}")
                        nc.vector.tensor_copy(otq[:], pb[q][:])
                        deng = nc.scalar if q % 2 == 0 else nc.sync
                        deng.dma_start(
                            outC[c * 128:(c + 1) * 128,
                                 q * 512:(q + 1) * 512], otq[:])
                    for j in range(KI):
                        nc.tensor.matmul(
                            pb3a[:], actT[c][:, j, :],
                            wd4[j // 4][:, j % 4, 1536:1920],
                            start=(j == 0), stop=(j == KI - 1),
                        )
                    ota = work.tile([128, 384], dt.bfloat16, tag="otl",
                                    bufs=4, name=f"ot{c}_3a")
                    nc.vector.tensor_copy(ota[:], pb3a[:])
                    nc.scalar.dma_start(
                        outC[c * 128:(c + 1) * 128, 1536:1920], ota[:])
                    for j in range(KI):
                        nc.tensor.matmul(
                            pb3b[:], actT[c][:, j, :],
                            wd4[j // 4][:, j % 4, 1920:2048],
                            start=(j == 0), stop=(j == KI - 1),
                        )
                    # own tag: a 5th "otl" alloc would reuse q0's buffer and
                    # stall on q0's DMA receipt (~1 us) right at the tail.
                    otb = work.tile([128, 128], dt.bfloat16, tag="otb",
                                    bufs=1, name=f"ot{c}_3b")
                    nc.vector.tensor_copy(otb[:], pb3b[:])
                    nc.sync.dma_start(
                        outC[c * 128:(c + 1) * 128, 1920:2048], otb[:])

    nc.compile()
    return nc


def _numpy_fallback(x, sel, scale, gate_up_weight, down_weight):
    """Correct host-side computation for overflow tokens (beyond the 512
    per-expert device capacity) and the pathological-skew full fallback."""
    wgu = np.asarray(gate_up_weight, dtype=np.float32)
    wdn = np.asarray(down_weight, dtype=np.float32)
    ii = wdn.shape[1]
    out = np.zeros_like(x)
    for e in range(wgu.shape[0]):
        tok = np.nonzero(sel == e)[0]
        if tok.size == 0:
            continue
        xsv = x[tok] * scale[tok][:, None]
        gu = xsv @ wgu[e]
        up, gate = gu[:, :ii], gu[:, ii:]
        out[tok] = (up * (gate / (1.0 + np.exp(-gate)))) @ wdn[e]
    return out


def kernel(hidden_states, gate_w, gate_up_weight, down_weight):
    from concourse.bass_utils import run_bass_kernel_spmd

    hs = np.asarray(hidden_states, dtype=np.float32)
    x = np.ascontiguousarray(hs).reshape(-1, H)
    nt = x.shape[0]
    gw = np.asarray(gate_w, dtype=np.float32)

    # Router (top-1): selected expert keeps sigmoid(logit), others contribute 0.
    logits = x @ gw.T                                   # [nt, E]
    sel = np.argmax(logits, axis=1)
    top = logits[np.arange(nt), sel]
    scale = (1.0 / (1.0 + np.exp(-top))).astype(np.float32)

    counts = np.bincount(sel, minlength=E)
    overflow = np.maximum(counts - C, 0)
    if int(overflow.sum()) > 1024:  # pathological skew; stay correct on host
        out = _numpy_fallback(x, sel, scale, gate_up_weight, down_weight)
        return out.reshape(hs.shape)
    counts_dev = np.minimum(counts, C)

    order = np.argsort(sel, kind="stable")
    offs = np.zeros(E + 1, dtype=np.int64)
    np.cumsum(counts, out=offs[1:])
    idx = np.zeros((E, C), dtype=np.int64)
    scale_pad = np.zeros((E, C), dtype=np.float32)
    over_tok = []
    for e in range(E):
        ce = int(counts_dev[e])
        idx[e, :ce] = order[offs[e]:offs[e] + ce]
        scale_pad[e, :ce] = scale[idx[e, :ce]]
        if int(counts[e]) > ce:
            over_tok.append(order[offs[e] + ce:offs[e] + int(counts[e])])

    gath = x[idx.reshape(-1)]                           # [E*C, H]
    gath *= scale_pad.reshape(-1, 1)
    gath_bf = gath.astype(ml_dtypes.bfloat16).reshape(E, C, H)
    # xsP[e, p, k*C + c] = x[c, k*128 + p]: partition-major (4 KB DMA rows)
    xsP_all = np.ascontiguousarray(
        gath_bf.reshape(E, C, KH, 128).transpose(0, 3, 2, 1)
    ).reshape(E, 128, KH * C)
    wgu_bf = np.asarray(gate_up_weight, dtype=np.float32).astype(ml_dtypes.bfloat16)
    # Pass-paired weight layouts (see _build_program): pass p streams the
    # (up q_p, gate q_{p+2}) column pair of every k-tile.
    w4 = wgu_bf.reshape(E, KH, 128, 4, 512)
    wguP0_all = np.ascontiguousarray(
        np.concatenate([w4[:, :, :, 0, :], w4[:, :, :, 2, :]], axis=-1))
    wguP1_all = np.ascontiguousarray(
        np.concatenate([w4[:, :, :, 1, :], w4[:, :, :, 3, :]], axis=-1)
        .transpose(0, 2, 1, 3)).reshape(E, 128, KH * 1024)
    # Packed gating pieces (see _build_program)
    xsW0_all = np.ascontiguousarray(np.concatenate(
        [xsP_all[:, :, 0:C], wguP0_all[:, 0, :, 0:512]], axis=-1))
    wg0b_all = np.ascontiguousarray(wguP0_all[:, 0, :, 512:1024])
    xsW1_all = np.ascontiguousarray(np.concatenate(
        [xsP_all[:, :, C:4 * C], wguP0_all[:, 1, :, :]], axis=-1))
    wdn_bf = np.asarray(down_weight, dtype=np.float32).astype(ml_dtypes.bfloat16)

    if "prog" not in _PROGRAM_CACHE:
        _PROGRAM_CACHE["prog"] = _build_program()
    nc = _PROGRAM_CACHE["prog"]
    in_maps = [
        {"xsP": xsP_all[e], "xsW0": xsW0_all[e], "wg0b": wg0b_all[e],
         "xsW1": xsW1_all[e], "wguP0": wguP0_all[e], "wguP1": wguP1_all[e],
         "wdn": wdn_bf[e]} for e in range(E)
    ]
    res = run_bass_kernel_spmd(nc, in_maps, list(range(E)))
    LAST["results"] = res
    LAST["C"] = C

    out = np.zeros((nt, H), dtype=np.float32)
    for e in range(E):
        ce = int(counts_dev[e])
        if ce:
            out[idx[e, :ce]] = res.results[e]["outC"][:ce, :].astype(np.float32)
    if over_tok:
        ov = np.concatenate(over_tok)
        out[ov] = _numpy_fallback(
            x[ov], sel[ov], scale[ov], gate_up_weight, down_weight)
    return out.reshape(hs.shape)


# revision 28
# speedup vs baseline: 1.0009x; 1.0009x over previous
"""Top-1 MoE (BmmMoeModel) on 8 Trainium2 NeuronCores.

Strategy: expert-parallel with routing-aware dispatch.
  - Host computes the router (x @ gate_w.T, argmax, sigmoid) -- 0.26% of the
    model FLOPs -- and uses it as the sharding function: each core receives
    only the tokens routed to its expert (scaled by the routing weight,
    transposed to [H, C], cast to bf16) plus that expert's weights.
  - The final "sum over experts" is a disjoint scatter of each core's token
    outputs back into the full [T, H] output on the host (top-1 routing means
    non-selected experts contribute exactly zero).

On-device schedule (all matmuls bf16, fp32 PSUM, stationary = activations,
moving = weights; at N=512 the PE streams 213 ns/matmul and LDWEIGHTS is
fully hidden by the reorder window, so the only wins left are at the edges):

  Phase A runs in TWO PASSES over (up, gate) column pairs.  Pass p streams
  weight columns [p*512:(p+1)*512] (up) and [1024+p*512:...] (gate) of every
  k-tile, pre-paired by the host into one contiguous [128, 1024] tile per
  (pass, k).  All four 128-token c-chunks accumulate over the FULL k=16
  contraction in PSUM (4c x 2 halves = 8 banks), so there is no partial-sum
  spill/merge at all, and the silu-glu fuses straight out of PSUM:
  act[c, pass-cols] = up * silu(gate) (one scalar.activation + one
  vector.mul).  Per arriving 0.25 MB weight tile the PE does 1.7 us of work
  vs 0.61 us arrival time, so the PE never starves once started (the old
  half-k/spill schedule consumed 1.7 us per 2.56-us-arriving tile during its
  first c-pair pass and idled ~5 us).

  act is transposed per pass to i-major actT with the DMA xbar (off the
  critical engines).  Phase B (down proj) is token-stationary: stationary =
  actT[i-chunk, c-chunk], moving = wdn[i-chunk, :] in 4 q-slices of 512,
  PSUM accumulates over i; output lands token-major [C, H] in bf16 (halves
  the drain tail; host upcasts).  The last c-chunk runs banks q0-q2 first so
  the post-final-matmul tail is a single 512-wide cast + 0.25 MB DMA.

DMA plan (measured: the NEFF preamble releases SWDGE/gpsimd at ~4.2 us but
HWDGE issuers only at ~7.0 us, and each dma_start occupies its issuing
engine ~0.6 us): the PE-gating pieces (xsA = k-tile 0 of tokens, the two
512-wide halves of pass-0 k-tile 0, xsB, pass-0 k-tiles 1-2) issue from
GPSIMD starting ~4.2 us, everything else from SCALAR in consumption order;
outputs ride sync/gpsimd.  The PE warmup spin (HAM clock gate holds the PE
at 1.2 GHz until ~3.4 us of sustained activity) is sized to end right when
the first weights land.
"""

import numpy as np
import ml_dtypes

B, S, H, I, E = 2, 2048, 2048, 1024, 8
T = B * S
FF2 = 2 * I
C = 512          # per-expert device token capacity (4 c-chunks of 128)
KH = H // 128    # 16 contraction chunks for gate_up
KI = I // 128    # 8 contraction chunks for down

# Stash of the last run's BassKernelResults (for test harness introspection).
LAST = {}
_PROGRAM_CACHE = {}


def _build_program():
    import concourse.bass as bass
    import concourse.mybir as mybir
    import concourse.tile as tile
    from concourse import bacc

    dt = mybir.dt
    AF = mybir.ActivationFunctionType

    nc = bacc.Bacc(None, target_bir_lowering=False)
    # xsP[p, k*C + c] = x_tokens[c, k*128 + p]: partition-major so each DMA
    # row run is 4 KB (k-group of 4) instead of 1 KB.
    xsP = nc.dram_tensor("xsP", [128, KH * C], dt.bfloat16,
                         kind="ExternalInput")
    # Packed PE-gating pieces (every dma_start costs ~0.65 us of
    # issuing-engine time and each tile completion pays ~0.5-1 us of
    # semaphore receipt, so the head wants FEW, SMALL, need-ordered pieces):
    #   xsW0 = [k-tile 0 of tokens | pass-0 k0 up cols]   -> first 4 matmuls
    #   wg0b = [pass-0 k0 gate cols]                      -> next 4
    #   xsW1 = [k-tiles 1-3 of tokens | pass-0 k1]        -> k=1 group
    xsW0 = nc.dram_tensor("xsW0", [128, 2 * 512], dt.bfloat16,
                          kind="ExternalInput")
    wg0b = nc.dram_tensor("wg0b", [128, 512], dt.bfloat16,
                          kind="ExternalInput")
    xsW1 = nc.dram_tensor("xsW1", [128, 5 * 512], dt.bfloat16,
                          kind="ExternalInput")
    # Pass-paired gate_up weights (host pre-arranged):
    #   wguP0[k, p, :512]  = W[k-tile, p,  q0-cols]   (up,   pass 0)
    #   wguP0[k, p, 512:]  = W[k-tile, p,  q2-cols]   (gate, pass 0)
    # wguP1 is the same for pass 1 (q1/q3) but partition-major so pass 1
    # streams as four 1 MB DMAs with 8 KB contiguous rows.
    wguP0 = nc.dram_tensor("wguP0", [KH, 128, 1024], dt.bfloat16,
                           kind="ExternalInput")
    wguP1 = nc.dram_tensor("wguP1", [128, KH * 1024], dt.bfloat16,
                           kind="ExternalInput")
    wdn = nc.dram_tensor("wdn", [I, H], dt.bfloat16, kind="ExternalInput")
    # Output in bf16: halves the output HBM traffic and the post-matmul
    # drain tail; the host upcasts (adds ~0.2% RMS, far inside the gate).
    outC = nc.dram_tensor("outC", [C, H], dt.bfloat16, kind="ExternalOutput")

    with tile.TileContext(nc) as tc:
        with (
            tc.tile_pool(name="res", bufs=1) as res,
            tc.tile_pool(name="work", bufs=1) as work,
            tc.tile_pool(name="psum", bufs=1, space=bass.MemorySpace.PSUM) as psum,
        ):
            # PE clock pre-warm (see module docstring).
            warm = work.tile([128, 128], dt.bfloat16, tag="warm", bufs=1,
                             name="warm")
            nc.vector.memset(warm[:], 0.0)
            warm_ps = psum.tile([128, 128], dt.float32, tag="ps", bufs=8,
                                name="warm_ps")
            for _ in range(27):
                nc.tensor.matmul(warm_ps[:], warm[:], warm[:],
                                 start=True, stop=True)

            pk0 = res.tile([128, 2, 512], dt.bfloat16, tag="pk0", name="pk0")
            pk0b = res.tile([128, 512], dt.bfloat16, tag="pk0b", name="pk0b")
            pk1 = res.tile([128, 5, 512], dt.bfloat16, tag="pk1", name="pk1")
            xs4 = [None] + [res.tile([128, 4, C], dt.bfloat16, tag=f"xs{b}",
                                     name=f"xs{b}") for b in range(1, 4)]
            wgp0 = [None, None] + [
                res.tile([128, 1024], dt.bfloat16, tag=f"wgp0{k}",
                         name=f"wgp0{k}") for k in range(2, KH)]
            wgp1 = [res.tile([128, 4, 1024], dt.bfloat16, tag=f"wgp1{b}",
                             name=f"wgp1{b}") for b in range(4)]
            xsP_r = xsP.rearrange("p (b j c) -> b p j c", b=4, j=4)
            xsW0_r = xsW0.rearrange("p (j c) -> p j c", j=2)
            xsW1_r = xsW1.rearrange("p (j c) -> p j c", j=5)
            wguP1_r = wguP1.rearrange("p (b j f) -> b p j f", b=4, j=4)

            # ONE input stream on SCALAR (HWDGE), in exact consumption order
            # (strict FIFO keeps arrival == need order at full bandwidth; a
            # second concurrent input queue just splits bandwidth between
            # tiles needed now and tiles needed later).
            nc.scalar.dma_start(pk0[:], xsW0_r)
            nc.scalar.dma_start(pk0b[:], wg0b[0:128, :])
            nc.scalar.dma_start(pk1[:, 0:3, :], xsW1_r[:, 0:3, :])
            nc.scalar.dma_start(pk1[:, 3:5, :], xsW1_r[:, 3:5, :])
            nc.scalar.dma_start(wgp0[2][:], wguP0[2])
            nc.scalar.dma_start(xs4[1][:], xsP_r[1])
            for k in range(3, 8):
                nc.scalar.dma_start(wgp0[k][:], wguP0[k])
            nc.scalar.dma_start(xs4[2][:], xsP_r[2])
            nc.scalar.dma_start(xs4[3][:], xsP_r[3])
            for k in range(8, KH):
                nc.scalar.dma_start(wgp0[k][:], wguP0[k])
            for b in range(4):
                nc.scalar.dma_start(wgp1[b][:], wguP1_r[b])
            # down weight (Phase B moving): 2 tiles of 4 i-chunks each.
            wd4 = []
            wdn_r = wdn.rearrange("(b j p) h -> b p j h", j=4, p=128)
            for b in range(2):
                t_ = res.tile([128, 4, H], dt.bfloat16, tag=f"wd{b}",
                              name=f"wd{b}")
                nc.scalar.dma_start(t_[:], wdn_r[b])
                wd4.append(t_)

            def xs_sl(k, c):
                if k == 0:
                    return pk0[:, 0, c * 128:(c + 1) * 128]
                if k < 4:
                    return pk1[:, k - 1, c * 128:(c + 1) * 128]
                return xs4[k // 4][:, k % 4, c * 128:(c + 1) * 128]

            act = [res.tile([128, I], dt.bfloat16, tag=f"act{c}",
                            name=f"act{c}") for c in range(4)]
            # actT[c][:, j, :] = act[c][:, j*128:(j+1)*128].T  (i-major)
            actT = [res.tile([128, KI, 128], dt.bfloat16, tag=f"actT{c}",
                             name=f"actT{c}") for c in range(4)]

            # Phase A: two passes, full-k PSUM accumulation, fused silu-glu.
            # The last SG k-steps run per-c so each chunk's consume (the
            # bank-freeing silu+mul) overlaps the next chunk's k-tail.
            SG = 3
            for ps in range(2):
                pa = {(c, h): psum.tile([128, 512], dt.float32, tag="ps",
                                        bufs=8, name=f"pa{ps}_{c}_{h}")
                      for c in range(4) for h in range(2)}

                def wg_sl(k, h, ps=ps):
                    if ps == 0:
                        if k == 0:
                            return pk0[:, 1, :] if h == 0 else pk0b[:]
                        if k == 1:
                            return pk1[:, 3 + h, :]
                        return wgp0[k][:, h * 512:(h + 1) * 512]
                    return wgp1[k // 4][:, k % 4, h * 512:(h + 1) * 512]

                def a_mm(k, c, pa=pa, wg_sl=wg_sl):
                    lhs = xs_sl(k, c)
                    for h in range(2):
                        nc.tensor.matmul(
                            pa[(c, h)][:], lhs, wg_sl(k, h),
                            start=(k == 0), stop=(k == KH - 1),
                        )

                def consume(c, pa=pa, ps=ps):
                    st = work.tile([128, 512], dt.float32, tag="silu",
                                   bufs=4, name=f"st{ps}_{c}")
                    nc.scalar.activation(st[:], pa[(c, 1)][:], AF.Silu)
                    nc.vector.tensor_mul(
                        act[c][:, ps * 512:(ps + 1) * 512],
                        pa[(c, 0)][:], st[:])
                    nc.sync.dma_start_transpose(
                        actT[c][:, ps * 4:(ps + 1) * 4, :],
                        act[c][:, ps * 512:(ps + 1) * 512])

                for k in range(0, KH - SG):
                    if k == 0:
                        # h0 for all c first: the h1 weight piece is still
                        # in flight when the first matmuls start.
                        for h in range(2):
                            for c in range(4):
                                nc.tensor.matmul(
                                    pa[(c, h)][:], xs_sl(0, c), wg_sl(0, h),
                                    start=True, stop=False)
                    else:
                        for c in range(4):
                            a_mm(k, c)
                for c in range(4):
                    for k in range(KH - SG, KH):
                        a_mm(k, c)
                    consume(c)

            # Phase B: token-major out, one c-chunk at a time (4 banks).
            for c in range(4):
                if c < 3:
                    pb = [psum.tile([128, 512], dt.float32, tag="ps", bufs=8,
                                    name=f"pb{c}_{q}") for q in range(4)]
                    for j in range(KI):
                        lhs = actT[c][:, j, :]
                        for q in range(4):
                            nc.tensor.matmul(
                                pb[q][:], lhs,
                                wd4[j // 4][:, j % 4, q * 512:(q + 1) * 512],
                                start=(j == 0), stop=(j == KI - 1),
                            )
                    ot = work.tile([128, H], dt.bfloat16, tag="ot", bufs=2,
                                   name=f"ot{c}")
                    for q in range(4):
                        nc.vector.tensor_copy(ot[:, q * 512:(q + 1) * 512],
                                              pb[q][:])
                    # alternate output queues so the final drains overlap
                    eng = nc.gpsimd if c % 2 == 0 else nc.sync
                    eng.dma_start(outC[c * 128:(c + 1) * 128, :], ot[:])
                else:
                    # banks q0-q2 interleaved (stationary reused), then the
                    # last 512 columns in two pieces (384 + 128): q0-q2's
                    # casts+DMAs overlap the 384-piece's matmuls, the
                    # 384-piece's drain overlaps the 128-piece's matmuls,
                    # and the post-final-matmul tail is one 64 KB DMA.
                    pb = [psum.tile([128, 512], dt.float32, tag="ps", bufs=8,
                                    name=f"pb{c}_{q}") for q in range(3)]
                    pb3a = psum.tile([128, 384], dt.float32, tag="ps", bufs=8,
                                     name=f"pb{c}_3a")
                    pb3b = psum.tile([128, 128], dt.float32, tag="ps", bufs=8,
                                     name=f"pb{c}_3b")
                    for j in range(KI):
                        lhs = actT[c][:, j, :]
                        for q in range(3):
                            nc.tensor.matmul(
                                pb[q][:], lhs,
                                wd4[j // 4][:, j % 4, q * 512:(q + 1) * 512],
                                start=(j == 0), stop=(j == KI - 1),
                            )
                    for q in range(3):
                        otq = work.tile([128, 512], dt.bfloat16, tag="otl",
                                        bufs=4, name=f"ot{c}_{q}")
                        nc.vector.tensor_copy(otq[:], pb[q][:])
                        deng = nc.scalar if q % 2 == 0 else nc.sync
                        deng.dma_start(
                            outC[c * 128:(c + 1) * 128,
                                 q * 512:(q + 1) * 512], otq[:])
                    for j in range(KI):
                        nc.tensor.matmul(
                            pb3a[:], actT[c][:, j, :],
                            wd4[j // 4][:, j % 4, 1536:1920],
                            start=(j == 0), stop=(j == KI - 1),
                        )
                    ota = work.tile([128, 384], dt.bfloat16, tag="otl",
                                    bufs=4, name=f"ot{c}_3a")
                    nc.vector.tensor_copy(ota[:], pb3a[:])
                    nc.scalar.dma_start(
                        outC[c * 128:(c + 1) * 128, 1536:1920], ota[:])
                    for j in range(KI):
                        nc.tensor.matmul(
                            pb3b[:], actT[c][:, j, :],
                            wd4[j // 4][:, j % 4, 1920:2048],
                            start=(j == 0), stop=(j == KI - 1),
                        )
                    # own tag: a 5th "otl" alloc would reuse q0's buffer and
                    # stall on q0's DMA receipt (~1 us) right at the tail.
                    otb = work.tile([128, 128], dt.bfloat16, tag="otb",
                                    bufs=1, name=f"ot{c}_3b")
                    nc.vector.tensor_copy(otb[:], pb3b[:])
                    nc.sync.dma_start(
                        outC[c * 128:(c + 1) * 128, 1920:2048], otb[:])

    nc.compile()
    return nc


def _numpy_fallback(x, sel, scale, gate_up_weight, down_weight):
    """Correct host-side computation for overflow tokens (beyond the 512
    per-expert device capacity) and the pathological-skew full fallback."""
    wgu = np.asarray(gate_up_weight, dtype=np.float32)
    wdn = np.asarray(down_weight, dtype=np.float32)
    ii = wdn.shape[1]
    out = np.zeros_like(x)
    for e in range(wgu.shape[0]):
        tok = np.nonzero(sel == e)[0]
        if tok.size == 0:
            continue
        xsv = x[tok] * scale[tok][:, None]
        gu = xsv @ wgu[e]
        up, gate = gu[:, :ii], gu[:, ii:]
        out[tok] = (up * (gate / (1.0 + np.exp(-gate)))) @ wdn[e]
    return out


def kernel(hidden_states, gate_w, gate_up_weight, down_weight):
    from concourse.bass_utils import run_bass_kernel_spmd

    hs = np.asarray(hidden_states, dtype=np.float32)
    x = np.ascontiguousarray(hs).reshape(-1, H)
    nt = x.shape[0]
    gw = np.asarray(gate_w, dtype=np.float32)

    # Router (top-1): selected expert keeps sigmoid(logit), others contribute 0.
    logits = x @ gw.T                                   # [nt, E]
    sel = np.argmax(logits, axis=1)
    top = logits[np.arange(nt), sel]
    scale = (1.0 / (1.0 + np.exp(-top))).astype(np.float32)

    counts = np.bincount(sel, minlength=E)
    overflow = np.maximum(counts - C, 0)
    if int(overflow.sum()) > 1024:  # pathological skew; stay correct on host
        out = _numpy_fallback(x, sel, scale, gate_up_weight, down_weight)
        return out.reshape(hs.shape)
    counts_dev = np.minimum(counts, C)

    order = np.argsort(sel, kind="stable")
    offs = np.zeros(E + 1, dtype=np.int64)
    np.cumsum(counts, out=offs[1:])
    idx = np.zeros((E, C), dtype=np.int64)
    scale_pad = np.zeros((E, C), dtype=np.float32)
    over_tok = []
    for e in range(E):
        ce = int(counts_dev[e])
        idx[e, :ce] = order[offs[e]:offs[e] + ce]
        scale_pad[e, :ce] = scale[idx[e, :ce]]
        if int(counts[e]) > ce:
            over_tok.append(order[offs[e] + ce:offs[e] + int(counts[e])])

    gath = x[idx.reshape(-1)]                           # [E*C, H]
    gath *= scale_pad.reshape(-1, 1)
    gath_bf = gath.astype(ml_dtypes.bfloat16).reshape(E, C, H)
    # xsP[e, p, k*C + c] = x[c, k*128 + p]: partition-major (4 KB DMA rows)
    xsP_all = np.ascontiguousarray(
        gath_bf.reshape(E, C, KH, 128).transpose(0, 3, 2, 1)
    ).reshape(E, 128, KH * C)
    wgu_bf = np.asarray(gate_up_weight, dtype=np.float32).astype(ml_dtypes.bfloat16)
    # Pass-paired weight layouts (see _build_program): pass p streams the
    # (up q_p, gate q_{p+2}) column pair of every k-tile.
    w4 = wgu_bf.reshape(E, KH, 128, 4, 512)
    wguP0_all = np.ascontiguousarray(
        np.concatenate([w4[:, :, :, 0, :], w4[:, :, :, 2, :]], axis=-1))
    wguP1_all = np.ascontiguousarray(
        np.concatenate([w4[:, :, :, 1, :], w4[:, :, :, 3, :]], axis=-1)
        .transpose(0, 2, 1, 3)).reshape(E, 128, KH * 1024)
    # Packed gating pieces (see _build_program)
    xsW0_all = np.ascontiguousarray(np.concatenate(
        [xsP_all[:, :, 0:C], wguP0_all[:, 0, :, 0:512]], axis=-1))
    wg0b_all = np.ascontiguousarray(wguP0_all[:, 0, :, 512:1024])
    xsW1_all = np.ascontiguousarray(np.concatenate(
        [xsP_all[:, :, C:4 * C], wguP0_all[:, 1, :, :]], axis=-1))
    wdn_bf = np.asarray(down_weight, dtype=np.float32).astype(ml_dtypes.bfloat16)

    if "prog" not in _PROGRAM_CACHE:
        _PROGRAM_CACHE["prog"] = _build_program()
    nc = _PROGRAM_CACHE["prog"]
    in_maps = [
        {"xsP": xsP_all[e], "xsW0": xsW0_all[e], "wg0b": wg0b_all[e],
         "xsW1": xsW1_all[e], "wguP0": wguP0_all[e], "wguP1": wguP1_all[e],
         "wdn": wdn_bf[e]} for e in range(E)
    ]
    res = run_bass_kernel_spmd(nc, in_maps, list(range(E)))
    LAST["results"] = res
    LAST["C"] = C

    out = np.zeros((nt, H), dtype=np.float32)
    for e in range(E):
        ce = int(counts_dev[e])
        if ce:
            out[idx[e, :ce]] = res.results[e]["outC"][:ce, :].astype(np.float32)
    if over_tok:
        ov = np.concatenate(over_tok)
        out[ov] = _numpy_fallback(
            x[ov], sel[ov], scale[ov], gate_up_weight, down_weight)
    return out.reshape(hs.shape)


# revision 35
# speedup vs baseline: 1.0260x; 1.0251x over previous
"""Top-1 MoE (BmmMoeModel) on 8 Trainium2 NeuronCores.

Strategy: expert-parallel with routing-aware dispatch.
  - Host computes the router (x @ gate_w.T, argmax, sigmoid) -- 0.26% of the
    model FLOPs -- and uses it as the sharding function: each core receives
    only the tokens routed to its expert (scaled by the routing weight,
    transposed to [H, C], cast to bf16) plus that expert's weights.
  - The final "sum over experts" is a disjoint scatter of each core's token
    outputs back into the full [T, H] output on the host (top-1 routing means
    non-selected experts contribute exactly zero).

On-device schedule (all matmuls bf16, fp32 PSUM, stationary = activations,
moving = weights; at N=512 the PE streams 213 ns/matmul and LDWEIGHTS is
fully hidden by the reorder window, so the only wins left are at the edges):

  Phase A runs in TWO PASSES over (up, gate) column pairs.  Pass p streams
  weight columns [p*512:(p+1)*512] (up) and [1024+p*512:...] (gate) of every
  k-tile, pre-paired by the host into one contiguous [128, 1024] tile per
  (pass, k).  All four 128-token c-chunks accumulate over the FULL k=16
  contraction in PSUM (4c x 2 halves = 8 banks), so there is no partial-sum
  spill/merge at all, and the silu-glu fuses straight out of PSUM:
  act[c, pass-cols] = up * silu(gate) (one scalar.activation + one
  vector.mul).  Per arriving 0.25 MB weight tile the PE does 1.7 us of work
  vs 0.61 us arrival time, so the PE never starves once started (the old
  half-k/spill schedule consumed 1.7 us per 2.56-us-arriving tile during its
  first c-pair pass and idled ~5 us).

  act is transposed per pass to i-major actT with the DMA xbar (off the
  critical engines).  Phase B (down proj) is token-stationary: stationary =
  actT[i-chunk, c-chunk], moving = wdn[i-chunk, :] in 4 q-slices of 512,
  PSUM accumulates over i; output lands token-major [C, H] in bf16 (halves
  the drain tail; host upcasts).  The last c-chunk runs banks q0-q2 first so
  the post-final-matmul tail is a single 512-wide cast + 0.25 MB DMA.

DMA plan (measured: the NEFF preamble releases SWDGE/gpsimd at ~4.2 us but
HWDGE issuers only at ~7.0 us, and each dma_start occupies its issuing
engine ~0.6 us): the PE-gating pieces (xsA = k-tile 0 of tokens, the two
512-wide halves of pass-0 k-tile 0, xsB, pass-0 k-tiles 1-2) issue from
GPSIMD starting ~4.2 us, everything else from SCALAR in consumption order;
outputs ride sync/gpsimd.  The PE warmup spin (HAM clock gate holds the PE
at 1.2 GHz until ~3.4 us of sustained activity) is sized to end right when
the first weights land.
"""

import numpy as np
import ml_dtypes

B, S, H, I, E = 2, 2048, 2048, 1024, 8
T = B * S
FF2 = 2 * I
C = 512          # per-expert device token capacity (4 c-chunks of 128)
KH = H // 128    # 16 contraction chunks for gate_up
KI = I // 128    # 8 contraction chunks for down

# Stash of the last run's BassKernelResults (for test harness introspection).
LAST = {}
_PROGRAM_CACHE = {}


def _build_program():
    import concourse.bass as bass
    import concourse.mybir as mybir
    import concourse.tile as tile
    from concourse import bacc

    dt = mybir.dt
    AF = mybir.ActivationFunctionType

    nc = bacc.Bacc(None, target_bir_lowering=False)
    # xsP[p, k*C + c] = x_tokens[c, k*128 + p]: partition-major so each DMA
    # row run is 4 KB (k-group of 4) instead of 1 KB.
    xsP = nc.dram_tensor("xsP", [128, KH * C], dt.bfloat16,
                         kind="ExternalInput")
    # Packed PE-gating pieces (every dma_start costs ~0.65 us of
    # issuing-engine time and each tile completion pays ~0.5-1 us of
    # semaphore receipt, so the head wants FEW, SMALL, need-ordered pieces):
    #   xsW0 = [k-tile 0 of tokens | pass-0 k0 up cols]   -> first 4 matmuls
    #   wg0b = [pass-0 k0 gate cols]                      -> next 4
    #   xsW1 = [k-tiles 1-3 of tokens | pass-0 k1]        -> k=1 group
    xsW0 = nc.dram_tensor("xsW0", [128, 2 * 512], dt.bfloat16,
                          kind="ExternalInput")
    wg0b = nc.dram_tensor("wg0b", [128, 512], dt.bfloat16,
                          kind="ExternalInput")
    # Pass-paired gate_up weights (host pre-arranged):
    #   wguP0[k, p, :512]  = W[k-tile, p,  q0-cols]   (up,   pass 0)
    #   wguP0[k, p, 512:]  = W[k-tile, p,  q2-cols]   (gate, pass 0)
    # wguP1 is the same for pass 1 (q1/q3) but partition-major so pass 1
    # streams as four 1 MB DMAs with 8 KB contiguous rows.
    wguP0 = nc.dram_tensor("wguP0", [KH, 128, 1024], dt.bfloat16,
                           kind="ExternalInput")
    wguP1 = nc.dram_tensor("wguP1", [128, KH * 1024], dt.bfloat16,
                           kind="ExternalInput")
    wdn = nc.dram_tensor("wdn", [I, H], dt.bfloat16, kind="ExternalInput")
    # Output in bf16: halves the output HBM traffic and the post-matmul
    # drain tail; the host upcasts (adds ~0.2% RMS, far inside the gate).
    outC = nc.dram_tensor("outC", [C, H], dt.bfloat16, kind="ExternalOutput")

    with tile.TileContext(nc) as tc:
        with (
            tc.tile_pool(name="res", bufs=1) as res,
            tc.tile_pool(name="work", bufs=1) as work,
            tc.tile_pool(name="psum", bufs=1, space=bass.MemorySpace.PSUM) as psum,
        ):
            # PE clock pre-warm (see module docstring).
            warm = work.tile([128, 128], dt.bfloat16, tag="warm", bufs=1,
                             name="warm")
            nc.vector.memset(warm[:], 0.0)
            warm_ps = psum.tile([128, 128], dt.float32, tag="ps", bufs=8,
                                name="warm_ps")
            for _ in range(27):
                nc.tensor.matmul(warm_ps[:], warm[:], warm[:],
                                 start=True, stop=True)

            pk0 = res.tile([128, 2, 512], dt.bfloat16, tag="pk0", name="pk0")
            pk0b = res.tile([128, 512], dt.bfloat16, tag="pk0b", name="pk0b")
            xsB = res.tile([128, 3, C], dt.bfloat16, tag="xsB", name="xsB")
            xs4 = [None] + [res.tile([128, 4, C], dt.bfloat16, tag=f"xs{b}",
                                     name=f"xs{b}") for b in range(1, 4)]
            # pass-0 weights land as SEPARATE half-tiles (0.125 MB each): in
            # the first ~15 us the 8 cores saturate their shared HBM domains
            # and the per-core stream runs at only ~0.25-0.31 MB/us vs the
            # PE's 0.22 MB/us need rate, so arrival granularity sets the
            # stall pattern -- half-tiles turn rare multi-us stalls (which
            # can trip a HAM re-throttle) into sub-us waits.
            wgh = {(k, h): res.tile([128, 512], dt.bfloat16, tag=f"wgh{k}_{h}",
                                    name=f"wgh{k}_{h}")
                   for k in range(1, KH) for h in range(2)}
            wgp1 = [res.tile([128, 4, 1024], dt.bfloat16, tag=f"wgp1{b}",
                             name=f"wgp1{b}") for b in range(4)]
            xsP_r = xsP.rearrange("p (b j c) -> b p j c", b=4, j=4)
            xsW0_r = xsW0.rearrange("p (j c) -> p j c", j=2)
            wguP1_r = wguP1.rearrange("p (b j f) -> b p j f", b=4, j=4)

            # ONE input stream on SCALAR (HWDGE), in exact consumption order
            # (strict FIFO keeps arrival == need order at full bandwidth; a
            # second concurrent input queue just splits bandwidth between
            # tiles needed now and tiles needed later).
            def wgh_dma(k):
                for h in range(2):
                    nc.scalar.dma_start(wgh[(k, h)][:],
                                        wguP0[k][:, h * 512:(h + 1) * 512])

            nc.scalar.dma_start(pk0[:], xsW0_r)
            nc.scalar.dma_start(pk0b[:], wg0b[0:128, :])
            nc.scalar.dma_start(xsB[:], xsP_r[0][:, 1:4, :])
            for k in range(1, 3):
                wgh_dma(k)
            nc.scalar.dma_start(xs4[1][:], xsP_r[1])
            for k in range(3, 8):
                wgh_dma(k)
            nc.scalar.dma_start(xs4[2][:], xsP_r[2])
            nc.scalar.dma_start(xs4[3][:], xsP_r[3])
            for k in range(8, KH):
                wgh_dma(k)
            for b in range(4):
                nc.scalar.dma_start(wgp1[b][:], wguP1_r[b])
            # down weight (Phase B moving): 2 tiles of 4 i-chunks each.
            wd4 = []
            wdn_r = wdn.rearrange("(b j p) h -> b p j h", j=4, p=128)
            for b in range(2):
                t_ = res.tile([128, 4, H], dt.bfloat16, tag=f"wd{b}",
                              name=f"wd{b}")
                nc.scalar.dma_start(t_[:], wdn_r[b])
                wd4.append(t_)

            def xs_sl(k, c):
                if k == 0:
                    return pk0[:, 0, c * 128:(c + 1) * 128]
                if k < 4:
                    return xsB[:, k - 1, c * 128:(c + 1) * 128]
                return xs4[k // 4][:, k % 4, c * 128:(c + 1) * 128]

            act = [res.tile([128, I], dt.bfloat16, tag=f"act{c}",
                            name=f"act{c}") for c in range(4)]
            # actT[c][:, j, :] = act[c][:, j*128:(j+1)*128].T  (i-major)
            actT = [res.tile([128, KI, 128], dt.bfloat16, tag=f"actT{c}",
                             name=f"actT{c}") for c in range(4)]

            # Phase A: two passes, full-k PSUM accumulation, fused silu-glu.
            # The last SG k-steps run per-c so each chunk's consume (the
            # bank-freeing silu+mul) overlaps the next chunk's k-tail.
            SG = 3
            for ps in range(2):
                pa = {(c, h): psum.tile([128, 512], dt.float32, tag="ps",
                                        bufs=8, name=f"pa{ps}_{c}_{h}")
                      for c in range(4) for h in range(2)}

                def wg_sl(k, h, ps=ps):
                    if ps == 0:
                        if k == 0:
                            return pk0[:, 1, :] if h == 0 else pk0b[:]
                        return wgh[(k, h)][:]
                    return wgp1[k // 4][:, k % 4, h * 512:(h + 1) * 512]

                def a_mm(k, c, pa=pa, wg_sl=wg_sl):
                    lhs = xs_sl(k, c)
                    for h in range(2):
                        nc.tensor.matmul(
                            pa[(c, h)][:], lhs, wg_sl(k, h),
                            start=(k == 0), stop=(k == KH - 1),
                        )

                def consume(c, pa=pa, ps=ps):
                    st = work.tile([128, 512], dt.float32, tag="silu",
                                   bufs=4, name=f"st{ps}_{c}")
                    nc.scalar.activation(st[:], pa[(c, 1)][:], AF.Silu)
                    nc.vector.tensor_mul(
                        act[c][:, ps * 512:(ps + 1) * 512],
                        pa[(c, 0)][:], st[:])
                    nc.sync.dma_start_transpose(
                        actT[c][:, ps * 4:(ps + 1) * 4, :],
                        act[c][:, ps * 512:(ps + 1) * 512])

                for k in range(0, KH - SG):
                    if ps == 0:
                        # h-outer: each 0.125 MB weight half-tile unlocks 4
                        # matmuls as soon as it lands (see wgh comment).
                        for h in range(2):
                            for c in range(4):
                                nc.tensor.matmul(
                                    pa[(c, h)][:], xs_sl(k, c), wg_sl(k, h),
                                    start=(k == 0), stop=False)
                    else:
                        for c in range(4):
                            a_mm(k, c)
                for c in range(4):
                    for k in range(KH - SG, KH):
                        a_mm(k, c)
                    consume(c)

            # Phase B: token-major out, one c-chunk at a time (4 banks).
            for c in range(4):
                if c < 3:
                    pb = [psum.tile([128, 512], dt.float32, tag="ps", bufs=8,
                                    name=f"pb{c}_{q}") for q in range(4)]
                    for j in range(KI):
                        lhs = actT[c][:, j, :]
                        for q in range(4):
                            nc.tensor.matmul(
                                pb[q][:], lhs,
                                wd4[j // 4][:, j % 4, q * 512:(q + 1) * 512],
                                start=(j == 0), stop=(j == KI - 1),
                            )
                    ot = work.tile([128, H], dt.bfloat16, tag="ot", bufs=2,
                                   name=f"ot{c}")
                    for q in range(4):
                        nc.vector.tensor_copy(ot[:, q * 512:(q + 1) * 512],
                                              pb[q][:])
                    # alternate output queues so the final drains overlap
                    eng = nc.gpsimd if c % 2 == 0 else nc.sync
                    eng.dma_start(outC[c * 128:(c + 1) * 128, :], ot[:])
                else:
                    # banks q0-q2 interleaved (stationary reused), then the
                    # last 512 columns in two pieces (384 + 128): q0-q2's
                    # casts+DMAs overlap the 384-piece's matmuls, the
                    # 384-piece's drain overlaps the 128-piece's matmuls,
                    # and the post-final-matmul tail is one 64 KB DMA.
                    pb = [psum.tile([128, 512], dt.float32, tag="ps", bufs=8,
                                    name=f"pb{c}_{q}") for q in range(3)]
                    pb3a = psum.tile([128, 384], dt.float32, tag="ps", bufs=8,
                                     name=f"pb{c}_3a")
                    pb3b = psum.tile([128, 128], dt.float32, tag="ps", bufs=8,
                                     name=f"pb{c}_3b")
                    for j in range(KI):
                        lhs = actT[c][:, j, :]
                        for q in range(3):
                            nc.tensor.matmul(
                                pb[q][:], lhs,
                                wd4[j // 4][:, j % 4, q * 512:(q + 1) * 512],
                                start=(j == 0), stop=(j == KI - 1),
                            )
                    for q in range(3):
                        otq = work.tile([128, 512], dt.bfloat16, tag="otl",
                                        bufs=4, name=f"ot{c}_{q}")
                        nc.vector.tensor_copy(otq[:], pb[q][:])
                        deng = nc.scalar if q % 2 == 0 else nc.sync
                        deng.dma_start(
                            outC[c * 128:(c + 1) * 128,
                                 q * 512:(q + 1) * 512], otq[:])
                    for j in range(KI):
                        nc.tensor.matmul(
                            pb3a[:], actT[c][:, j, :],
                            wd4[j // 4][:, j % 4, 1536:1920],
                            start=(j == 0), stop=(j == KI - 1),
                        )
                    ota = work.tile([128, 384], dt.bfloat16, tag="otl",
                                    bufs=4, name=f"ot{c}_3a")
                    nc.vector.tensor_copy(ota[:], pb3a[:])
                    nc.scalar.dma_start(
                        outC[c * 128:(c + 1) * 128, 1536:1920], ota[:])
                    for j in range(KI):
                        nc.tensor.matmul(
                            pb3b[:], actT[c][:, j, :],
                            wd4[j // 4][:, j % 4, 1920:2048],
                            start=(j == 0), stop=(j == KI - 1),
                        )
                    # own tag: a 5th "otl" alloc would reuse q0's buffer and
                    # stall on q0's DMA receipt (~1 us) right at the tail.
                    otb = work.tile([128, 128], dt.bfloat16, tag="otb",
                                    bufs=1, name=f"ot{c}_3b")
                    nc.vector.tensor_copy(otb[:], pb3b[:])
                    nc.sync.dma_start(
                        outC[c * 128:(c + 1) * 128, 1920:2048], otb[:])

    nc.compile()
    return nc


def _numpy_fallback(x, sel, scale, gate_up_weight, down_weight):
    """Correct host-side computation for overflow tokens (beyond the 512
    per-expert device capacity) and the pathological-skew full fallback."""
    wgu = np.asarray(gate_up_weight, dtype=np.float32)
    wdn = np.asarray(down_weight, dtype=np.float32)
    ii = wdn.shape[1]
    out = np.zeros_like(x)
    for e in range(wgu.shape[0]):
        tok = np.nonzero(sel == e)[0]
        if tok.size == 0:
            continue
        xsv = x[tok] * scale[tok][:, None]
        gu = xsv @ wgu[e]
        up, gate = gu[:, :ii], gu[:, ii:]
        out[tok] = (up * (gate / (1.0 + np.exp(-gate)))) @ wdn[e]
    return out


def kernel(hidden_states, gate_w, gate_up_weight, down_weight):
    from concourse.bass_utils import run_bass_kernel_spmd

    hs = np.asarray(hidden_states, dtype=np.float32)
    x = np.ascontiguousarray(hs).reshape(-1, H)
    nt = x.shape[0]
    gw = np.asarray(gate_w, dtype=np.float32)

    # Router (top-1): selected expert keeps sigmoid(logit), others contribute 0.
    logits = x @ gw.T                                   # [nt, E]
    sel = np.argmax(logits, axis=1)
    top = logits[np.arange(nt), sel]
    scale = (1.0 / (1.0 + np.exp(-top))).astype(np.float32)

    counts = np.bincount(sel, minlength=E)
    overflow = np.maximum(counts - C, 0)
    if int(overflow.sum()) > 1024:  # pathological skew; stay correct on host
        out = _numpy_fallback(x, sel, scale, gate_up_weight, down_weight)
        return out.reshape(hs.shape)
    counts_dev = np.minimum(counts, C)

    order = np.argsort(sel, kind="stable")
    offs = np.zeros(E + 1, dtype=np.int64)
    np.cumsum(counts, out=offs[1:])
    idx = np.zeros((E, C), dtype=np.int64)
    scale_pad = np.zeros((E, C), dtype=np.float32)
    over_tok = []
    for e in range(E):
        ce = int(counts_dev[e])
        idx[e, :ce] = order[offs[e]:offs[e] + ce]
        scale_pad[e, :ce] = scale[idx[e, :ce]]
        if int(counts[e]) > ce:
            over_tok.append(order[offs[e] + ce:offs[e] + int(counts[e])])

    gath = x[idx.reshape(-1)]                           # [E*C, H]
    gath *= scale_pad.reshape(-1, 1)
    gath_bf = gath.astype(ml_dtypes.bfloat16).reshape(E, C, H)
    # xsP[e, p, k*C + c] = x[c, k*128 + p]: partition-major (4 KB DMA rows)
    xsP_all = np.ascontiguousarray(
        gath_bf.reshape(E, C, KH, 128).transpose(0, 3, 2, 1)
    ).reshape(E, 128, KH * C)
    wgu_bf = np.asarray(gate_up_weight, dtype=np.float32).astype(ml_dtypes.bfloat16)
    # Pass-paired weight layouts (see _build_program): pass p streams the
    # (up q_p, gate q_{p+2}) column pair of every k-tile.
    w4 = wgu_bf.reshape(E, KH, 128, 4, 512)
    wguP0_all = np.ascontiguousarray(
        np.concatenate([w4[:, :, :, 0, :], w4[:, :, :, 2, :]], axis=-1))
    wguP1_all = np.ascontiguousarray(
        np.concatenate([w4[:, :, :, 1, :], w4[:, :, :, 3, :]], axis=-1)
        .transpose(0, 2, 1, 3)).reshape(E, 128, KH * 1024)
    # Packed gating pieces (see _build_program)
    xsW0_all = np.ascontiguousarray(np.concatenate(
        [xsP_all[:, :, 0:C], wguP0_all[:, 0, :, 0:512]], axis=-1))
    wg0b_all = np.ascontiguousarray(wguP0_all[:, 0, :, 512:1024])
    wdn_bf = np.asarray(down_weight, dtype=np.float32).astype(ml_dtypes.bfloat16)

    if "prog" not in _PROGRAM_CACHE:
        _PROGRAM_CACHE["prog"] = _build_program()
    nc = _PROGRAM_CACHE["prog"]
    in_maps = [
        {"xsP": xsP_all[e], "xsW0": xsW0_all[e], "wg0b": wg0b_all[e],
         "wguP0": wguP0_all[e], "wguP1": wguP1_all[e],
         "wdn": wdn_bf[e]} for e in range(E)
    ]
    res = run_bass_kernel_spmd(nc, in_maps, list(range(E)))
    LAST["results"] = res
    LAST["C"] = C

    out = np.zeros((nt, H), dtype=np.float32)
    for e in range(E):
        ce = int(counts_dev[e])
        if ce:
            out[idx[e, :ce]] = res.results[e]["outC"][:ce, :].astype(np.float32)
    if over_tok:
        ov = np.concatenate(over_tok)
        out[ov] = _numpy_fallback(
            x[ov], sel[ov], scale[ov], gate_up_weight, down_weight)
    return out.reshape(hs.shape)
